# revision 48
# baseline (speedup 1.0000x reference)
"""Trainium2 Bass kernel for conv->conv->self-attention->pool->fc classifier.

Shards batch 256 across 8 NeuronCores (32 samples each), weights replicated.
Heavy algebraic folding is done host-side (see _prep_consts); the device code
per sample is: 2 matmul convs + relu, one 65x65 "score operator" matmul,
4 QK^T score matmuls, exp with fused row-sum accumulation, reciprocal, a
recip-weighted column-sum matvec (w = E^T r), and a 10-wide output matvec.
The attn@V matmul is eliminated entirely: mean-pooling commutes with
attention, so only attention column sums (w) are needed.

PE packing: conv1 runs 4 samples concurrently via column tiling (outputs at
partition strips 32j of one PSUM bank); conv2 runs 2 samples concurrently via
row tiling (samples' h1 live at partition strips 32i of a shared tile).

Emission is stage-batched over groups of G samples, with each group's tail
stage (recip/w/logits, which depends on that group's exps) delayed by one
group so no engine queue ever stalls on a fresh dependency.
"""
import contextlib
import sys

sys.path.insert(0, "/opt/trn_rl_repo")

import numpy as np

import concourse.bass as bass
import concourse.bass_utils as _bass_utils
import concourse.tile as tile
from concourse import bacc, mybir
from concourse.bass_utils import run_bass_kernel_spmd

# The stock walrus invocation disables LDW dedup; consecutive matmuls here
# share stationary operands (S/G pairs), and enabling the optimization is
# measurably faster with bit-identical outputs on this kernel.
if not getattr(_bass_utils, "_ldw_opt_patched", False):
    _orig_run_command = _bass_utils.run_command

    def _run_command_ldw(argv, **kw):
        argv = ["--enable-ldw-opt=true" if a == "--enable-ldw-opt=false"
                else a for a in argv]
        return _orig_run_command(argv, **kw)

    _bass_utils.run_command = _run_command_ldw
    _bass_utils._ldw_opt_patched = True

# Problem constants (hardcoded per harness contract)
B, C_IN, L, NCLASS = 256, 6, 512, 10
NCORES = 8
BS = B // NCORES          # samples per core
C1, C2 = 32, 64           # conv output channels
K1 = 3 * C_IN + 1         # 19: im2col rows + ones row
DA = C2 + 1               # 65: augmented feature dim
DT = mybir.dt.float32
DTR = mybir.dt.float32r  # TF32-like PE fast path
EPS = 1e-5
G = 4                     # samples per pipeline group


def _prep_consts(p):
    """Fold all weights/biases/BN into the minimal set of device tensors."""
    inv1 = p["bn1_g"] / np.sqrt(p["bn1_v"] + EPS)            # [32]
    b1p = p["conv1_b"] * inv1 + p["bn1_b"] - p["bn1_m"] * inv1
    # W1p [19, 32]: rows t*6+c hold conv1_w[o,c,t]*inv1[o]; row 18 = fused bias
    w1p = np.zeros((K1, C1), np.float32)
    for t in range(3):
        w1p[t * C_IN:(t + 1) * C_IN, :] = (
            p["conv1_w"][:, :, t] * inv1[:, None]).T
    w1p[K1 - 1, :] = b1p

    inv2 = p["bn2_g"] / np.sqrt(p["bn2_v"] + EPS)            # [64]
    b2p = (p["conv2_b"] * inv2 + p["bn2_b"] - p["bn2_m"] * inv2).astype(
        np.float32).reshape(C2, 1)
    # W2 [32, 3*64] (tap-major columns), replicated on 4 partition strips so
    # row-tiled conv2 matmuls can contract any strip.
    w2 = np.concatenate([(p["conv2_w"][:, :, t] * inv2[:, None]).T
                         for t in range(3)], axis=1).astype(np.float32)
    w2rep = np.tile(w2, (4, 1))                              # [128, 192]

    wq, bq, wk, bk = p["wq"], p["bq"], p["wk"], p["bk"]
    maug = np.zeros((DA, DA), np.float32)
    maug[:C2, :C2] = wq.T @ wk
    maug[:C2, C2] = wq.T @ bk
    maug[C2, :C2] = wk.T @ bq
    maug[C2, C2] = float(bq @ bk)
    maug /= np.sqrt(64.0)
    maug_t = np.ascontiguousarray(maug.T)                    # lhsT for t-matmul

    # FaugT [65, 10]: G_t[k,c] = h_aug(k) . FaugT[:,c]
    #   rows 0-63 = (fc_w @ wv / 512).T ; row 64 = (fc_w @ bv + fc_b)/512
    # (row 64 exploits sum_k w[k] == 512 up to fp eps)
    faug_t = np.zeros((DA, NCLASS), np.float32)
    faug_t[:C2, :] = (p["fc_w"] @ p["wv"] / L).T
    faug_t[C2, :] = (p["fc_w"] @ p["bv"] + p["fc_b"]) / L
    return {
        "w1p": w1p.astype(np.float32),
        "w2rep": w2rep,
        "b2p": b2p,
        "maug_t": maug_t,
        "faug_t": faug_t,
    }


def _prep_x3(x_shard):
    """im2col with ones row: [BS,6,512] -> [19, BS*512] (fp32)."""
    bs = x_shard.shape[0]
    x3 = np.zeros((K1, bs, L), np.float32)
    x3[0:C_IN, :, 1:] = np.transpose(x_shard, (1, 0, 2))[:, :, :-1]
    x3[C_IN:2 * C_IN, :, :] = np.transpose(x_shard, (1, 0, 2))
    x3[2 * C_IN:3 * C_IN, :, :511] = np.transpose(x_shard, (1, 0, 2))[:, :, 1:]
    x3[K1 - 1, :, :] = 1.0
    return np.ascontiguousarray(x3.reshape(K1, bs * L))


def _build_program(repeat=1, dyn_loop=0):
    nc = bacc.Bacc("TRN2", target_bir_lowering=False, debug=False,
                   enable_asserts=True)
    x3_d = nc.dram_tensor("x3", [K1, BS * L], DT, kind="ExternalInput")
    w1p_d = nc.dram_tensor("w1p", [K1, C1], DT, kind="ExternalInput")
    w2_d = nc.dram_tensor("w2rep", [4 * C1, 3 * C2], DTR, kind="ExternalInput")
    b2p_d = nc.dram_tensor("b2p", [C2, 1], DT, kind="ExternalInput")
    maug_d = nc.dram_tensor("maug_t", [DA, DA], DTR, kind="ExternalInput")
    faug_d = nc.dram_tensor("faug_t", [DA, NCLASS], DTR, kind="ExternalInput")
    czero_d = nc.dram_tensor("czero", [128, 1], DTR, kind="ExternalInput")
    cone_d = nc.dram_tensor("cone", [1, L], DTR, kind="ExternalInput")
    out_d = nc.dram_tensor("out", [1, BS * NCLASS], DT, kind="ExternalOutput")

    with tile.TileContext(nc) as tc:
        with (
            nc.allow_low_precision(reason="float32r matmul fast path"),
            tc.tile_pool(name="consts", bufs=1) as consts,
            tc.tile_pool(name="persist", bufs=1) as persist,
            tc.tile_pool(name="tpool", bufs=3) as tpool,
            tc.tile_pool(name="epool", bufs=8 * G + 2) as epool,
            tc.tile_pool(name="small", bufs=3 * G + 2) as small,
            tc.tile_pool(name="ps_c1", bufs=1, space="PSUM") as ps_c1,
            tc.tile_pool(name="ps_c2", bufs=2, space="PSUM") as ps_c2,
            tc.tile_pool(name="ps_t", bufs=1, space="PSUM") as ps_t,
            tc.tile_pool(name="ps_s", bufs=2, space="PSUM") as ps_s,
            tc.tile_pool(name="ps_tl", bufs=2, space="PSUM") as ps_tl,
        ):
            w1p_t = consts.tile([K1, C1], DT)
            w2_t = consts.tile([4 * C1, 3 * C2], DTR)
            b2p_t = consts.tile([C2, 1], DT)
            maug_t = consts.tile([DA, DA], DTR)
            faug_t = consts.tile([DA, NCLASS], DTR)
            out_row = consts.tile([1, BS * NCLASS], DT)
            czero_t = consts.tile([128, 1], DTR)
            cone_t = consts.tile([1, L], DTR)

            # Startup DMAs ordered so group 0's critical inputs land first.
            x3bufs = [persist.tile([K1, G * L], DT, tag=f"x3_{i}",
                                   name=f"x3b_{i}")
                      for i in range(BS // G)]
            nc.sync.dma_start(x3bufs[0][:], x3_d.ap()[:, 0:G * L])
            nc.sync.dma_start(w1p_t[:], w1p_d.ap())
            nc.sync.dma_start(w2_t[:], w2_d.ap())
            nc.sync.dma_start(b2p_t[:], b2p_d.ap())
            nc.sync.dma_start(czero_t[:], czero_d.ap())
            nc.sync.dma_start(cone_t[:], cone_d.ap())
            nc.sync.dma_start(maug_t[:], maug_d.ap())
            nc.sync.dma_start(faug_t[:], faug_d.ap())

            # Persistent conv/h2 buffers: pad columns and the ones row are
            # written once here, then only the data regions are rewritten
            # per sample, avoiding per-sample memset/DMA traffic.
            N_H1, N_H2 = 4, 12
            h1bufs = []
            for i in range(N_H1):
                h1b = persist.tile([128, L + 2], DTR, tag=f"h1_{i}")
                nc.sync.dma_start(h1b[:, 0:1], czero_t[:])
                nc.sync.dma_start(h1b[:, L + 1:L + 2], czero_t[:])
                h1bufs.append(h1b)
            h2bufs = []
            for i in range(N_H2):
                h2b = persist.tile([DA, L], DTR, tag=f"h2_{i}")
                nc.sync.dma_start(h2b[C2:DA, :], cone_t[:])
                h2bufs.append(h2b)
            # remaining x3 slices (prefetched; spread across DMA queues)
            for i in range(1, BS // G):
                nc.sync.dma_start(
                    x3bufs[i][:], x3_d.ap()[:, i * G * L:(i + 1) * G * L])

            def head_stage(g0, state):
                """conv1/conv2 (packed) -> t -> S chunks + exp, for samples
                [g0, g0+G). Tail inputs are left in `state`."""
                x3g = x3bufs[g0 // G]
                # conv1: 4 samples concurrently via column tiling
                c1_p = ps_c1.tile([128, L], DT, tag="c1")
                for j in range(G):
                    nc.tensor.matmul(
                        c1_p[32 * j:32 * j + 32, :], w1p_t[:],
                        x3g[:, j * L:(j + 1) * L],
                        start=True, stop=True, tile_position=(0, 32 * j))
                h1pack = h1bufs[(g0 // G) % N_H1]
                nc.vector.tensor_scalar_max(h1pack[:, 1:L + 1], c1_p[:], 0.0)

                # conv2: pairs of samples via row tiling (strips 2p, 2p+1)
                c2_ps = []
                for p_ in range(2):
                    pa = ps_c2.tile([C2, L], DT, tag="c2")
                    pb = ps_c2.tile([C2, L], DT, tag="c2")
                    for t in range(3):
                        for half, cp in ((0, pa), (1, pb)):
                            st = 32 * (2 * p_ + half)
                            nc.tensor.matmul(
                                cp[:],
                                w2_t[st:st + 32,
                                     t * C2:(t + 1) * C2],
                                h1pack[st:st + 32, t:t + L],
                                start=(t == 0), stop=(t == 2),
                                tile_position=(st, 0))
                    c2_ps += [pa, pb]

                for j in range(G):
                    h2aug = h2bufs[(g0 + j) % N_H2]
                    nc.vector.tensor_scalar(
                        out=h2aug[0:C2, :], in0=c2_ps[j][:], scalar1=b2p_t[:],
                        scalar2=0.0, op0=mybir.AluOpType.add,
                        op1=mybir.AluOpType.max)

                    t_p = ps_t.tile([DA, L], DT, tag="tp")
                    nc.tensor.matmul(t_p[:], maug_t[:],
                                     h2aug[:],
                                     start=True, stop=True)
                    t_s = tpool.tile([DA, L], DTR, tag="ts")
                    nc.vector.tensor_copy(t_s[:], t_p[:])

                    zcol = small.tile([128, 4], DT, tag="z")
                    e_ts = []
                    for m in range(4):
                        s_p = ps_s.tile([128, L], DT, tag="sp")
                        nc.tensor.matmul(
                            s_p[:],
                            h2aug[:, m * 128:(m + 1) * 128],
                            t_s[:], start=True, stop=True)
                        e_t = epool.tile([128, L], DTR, tag="e")
                        nc.scalar.activation(
                            e_t[:], s_p[:],
                            mybir.ActivationFunctionType.Exp,
                            accum_out=zcol[:, m:m + 1])
                        e_ts.append(e_t)
                    # G_t depends only on h2aug; do it here so the tail (and
                    # especially the kernel endgame) stays short.
                    g_p = ps_tl.tile([128, 4 * NCLASS], DT, tag="tail")
                    for m in range(4):
                        nc.tensor.matmul(
                            g_p[:, m * NCLASS:(m + 1) * NCLASS],
                            h2aug[:, m * 128:(m + 1) * 128],
                            faug_t[:], start=True, stop=True)
                    g_s = small.tile([128, 4 * NCLASS], DTR, tag="gs")
                    nc.vector.tensor_copy(g_s[:], g_p[:])
                    state[g0 + j] = (g_s, zcol, e_ts)

            def tail_stage(samples, state):
                """recip -> w matvec -> G_t -> logits. Sub-stages are batched
                across the group so the in-order DVE/DMA queues never chain
                one sample's late ops before the next sample's early ops."""
                rc, wt, gs = {}, {}, {}
                for s in samples:
                    _, zcol, _ = state[s]
                    rcol = small.tile([128, 4], DTR, tag="r")
                    nc.vector.reciprocal(rcol[:], zcol[:])
                    rc[s] = rcol
                for s in samples:
                    _, _, e_ts = state[s]
                    w_p = ps_tl.tile([1, L], DT, tag="tail")
                    for m in range(4):
                        nc.tensor.matmul(w_p[:],
                                         rc[s][:, m:m + 1],
                                         e_ts[m][:],
                                         start=(m == 0), stop=(m == 3))
                    # evacuate interleaved: w_s[4*(k%128) + k//128] = w[k], so
                    # each partition's 4 values are contiguous and the
                    # transpose below is one 128-descriptor DMA.
                    w_s = small.tile([1, L], DTR, tag="ws")
                    nc.vector.tensor_copy(
                        w_s[0:1, :].rearrange("p (l h) -> p h l", h=4),
                        w_p[0:1, :].rearrange("p (h l) -> p h l", h=4))
                    # transpose w to partitions: w_t[p, m] = w[m*128 + p]
                    w_t = small.tile([128, 4], DTR, tag="wt")
                    nc.sync.dma_start(w_t[:, :], w_s[0:1, :])
                    wt[s] = w_t
                for s in samples:
                    gs[s] = state[s][0]
                for s in samples:
                    state.pop(s)
                    lg_p = ps_tl.tile([1, NCLASS], DT, tag="tail")
                    for m in range(4):
                        nc.tensor.matmul(
                            lg_p[:], wt[s][:, m:m + 1],
                            gs[s][:, m * NCLASS:(m + 1) * NCLASS],
                            start=(m == 0), stop=(m == 3))
                    nc.vector.tensor_copy(
                        out_row[0:1, s * NCLASS:(s + 1) * NCLASS], lg_p[:])

            loop_cm = (tc.For_i(0, dyn_loop, 1) if dyn_loop
                       else contextlib.nullcontext())
            with loop_cm:
                for _ in range(repeat):
                    state = {}
                    head_stage(0, state)
                    for g0 in range(G, BS, G):
                        head_stage(g0, state)
                        tail_stage(range(g0 - G, g0), state)
                    tail_stage(range(BS - G, BS), state)

            nc.sync.dma_start(out_d.ap(), out_row[:])

    nc.compile()
    return nc


_NC_CACHE = {}


def _get_program(repeat=1, dyn_loop=0):
    key = (repeat, dyn_loop)
    if key not in _NC_CACHE:
        _NC_CACHE[key] = _build_program(repeat, dyn_loop)
    return _NC_CACHE[key]


def kernel(**inputs):
    inputs = {k: np.asarray(v) for k, v in inputs.items()}
    consts = _prep_consts(inputs)
    x = inputs["x"].astype(np.float32)

    nc = _get_program()
    in_maps = []
    for i in range(NCORES):
        m = {"x3": _prep_x3(x[i * BS:(i + 1) * BS])}
        m.update({
            "czero": np.zeros((128, 1), np.float32),
            "cone": np.ones((1, L), np.float32),
            "w1p": consts["w1p"],
            "w2rep": consts["w2rep"],
            "b2p": consts["b2p"],
            "maug_t": consts["maug_t"],
            "faug_t": consts["faug_t"],
        })
        in_maps.append(m)
    res = run_bass_kernel_spmd(nc, in_maps, list(range(NCORES)))
    outs = [res.results[i]["out"].reshape(BS, NCLASS) for i in range(NCORES)]
    return np.concatenate(outs, axis=0)
